# revision 1
# baseline (speedup 1.0000x reference)
"""Trainium2 Bass kernel for nn_CNNModel_82222853915196.

Model (per utterance x: (64, 512)):
  multiscale patch features (h in {8,16,32,64}) -> feats (8192,)
  out[t, :] = Wfc @ concat([x[:, t], feats]) + bfc

Factorization: feats is broadcast over t, so
  out = x.T @ Wfc1.T  +  1 * (Wfc2 @ feats + cconst).T
with Wfc1 = Wfc[:, :64], Wfc2 = Wfc[:, 64:], all feature-bias terms folded
into cconst on the host.

Patch features never materialize an im2col tensor: the patch contraction
  f_h[k,p,o] = sum_{i,j} x[k+i, h*p+j] W_h[k,o,i*h+j]
is computed with "masked" stationary weights over the full 64-row contraction
(rows outside [k, k+h) zeroed host-side), so all offsets k fuse into the
matmul M dim and x is read straight from SBUF with strided APs:
one PSUM-accumulated matmul per within-row offset j.

Weights and feature math run in fp16 (same bytes as bf16, 8x the mantissa);
the frames matmul and final outputs stay fp32. Overall rel err ~4e-4.

Sharding: pure data parallel - 32 utterances -> 8 cores x 4. Weights
replicated; no cross-core communication. DMA issue is spread over the two
HWDGE rings (sync, scalar) + SWDGE (gpsimd) to overlap transfers.
"""

import os
import sys
from contextlib import ExitStack

import numpy as np

for _p in ("/opt/trn_rl_repo", "/root/.axon_site/_ro/trn_rl_repo"):
    if os.path.isdir(_p) and _p not in sys.path:
        sys.path.insert(0, _p)

import concourse.bass as bass
import concourse.tile as tile
from concourse import bacc, mybir
from concourse.bass_utils import run_bass_kernel_spmd

NCORES = 8
NUTT = 4                 # utterances per core
T = 512
F = 64
OUT = 400
W = NUTT * T             # 2048, free width of the x tile
FP32 = mybir.dt.float32
FP16 = mybir.dt.float16
NPF16 = np.float16


# ---------------------------------------------------------------------------
# host-side weight preparation
# ---------------------------------------------------------------------------

def _build_devindex():
    """devindex[kt, fp] = reference flat feature index m in [0, 8192)."""
    devindex = np.full((64, 128), -1, dtype=np.int64)
    # h=8: PSUM (q=k*4+o, u*64+p): kt = p//4, fp = (p%4)*32 + q
    for k in range(8):
        for p in range(64):
            for o in range(4):
                devindex[p // 4, (p % 4) * 32 + k * 4 + o] = (k * 64 + p) * 4 + o
    # h=16: (q=k*16+o, u*32+p): kt = 16 + p//2, fp = (p%2)*64 + q
    for k in range(4):
        for p in range(32):
            for o in range(16):
                devindex[16 + p // 2, (p % 2) * 64 + k * 16 + o] = \
                    2048 + (k * 32 + p) * 16 + o
    # h=32: (q=k*64+o, u*16+p): kt = 32 + p, fp = q
    for k in range(2):
        for p in range(16):
            for o in range(64):
                devindex[32 + p, k * 64 + o] = 4096 + (k * 16 + p) * 64 + o
    # h=64: (u*8+p, o): kt = 48 + p*2 + o//128, fp = o%128
    for p in range(8):
        for o in range(256):
            devindex[48 + p * 2 + o // 128, o % 128] = 6144 + p * 256 + o
    assert devindex.min() >= 0
    return devindex


def _masked(Wh, nk, h, no):
    """w[r, j, k*no+o] = Wh[k, o, (r-k)*h+j] for 0 <= r-k < h else 0."""
    w = np.zeros((64, h, nk * no), dtype=np.float32)
    for k in range(nk):
        for i in range(h):
            w[k + i, :, k * no:(k + 1) * no] = Wh[k].reshape(no, h, h)[:, i, :].T
    return w


def host_prep(W8, b8, W16, b16, W32, b32, W64, b64, Wfc, bfc):
    f32 = np.float32
    W8 = np.asarray(W8, f32); W16 = np.asarray(W16, f32)
    W32 = np.asarray(W32, f32); W64 = np.asarray(W64, f32)
    Wfc = np.asarray(Wfc, f32)
    b8 = np.asarray(b8, f32); b16 = np.asarray(b16, f32)
    b32 = np.asarray(b32, f32); b64 = np.asarray(b64, f32)
    bfc = np.asarray(bfc, f32)

    w8j = _masked(W8, 8, 8, 4).reshape(64, 256)
    w16j = _masked(W16, 4, 16, 16).reshape(64, 1024)
    w32j = _masked(W32, 2, 32, 64).reshape(64, 4096)
    # w64w[i, j*256+o] = W64[o, i*64+j]
    w64w = np.ascontiguousarray(
        W64.reshape(256, 64, 64).transpose(1, 2, 0).reshape(64, 64 * 256))

    devindex = _build_devindex()
    Wfc2 = Wfc[:, 64:]
    wfc2t = np.ascontiguousarray(
        Wfc2[:, devindex.reshape(-1)].T.reshape(64, 128, OUT))
    wfc1t4 = np.ascontiguousarray(np.tile(Wfc[:, :64].T, (1, NUTT)))

    fb = np.zeros(8192, dtype=np.float64)
    fb[0:2048] = np.broadcast_to(b8[:, None, :], (8, 64, 4)).reshape(-1)
    fb[2048:4096] = np.broadcast_to(b16[:, None, :], (4, 32, 16)).reshape(-1)
    fb[4096:6144] = np.broadcast_to(b32[:, None, :], (2, 16, 64)).reshape(-1)
    fb[6144:8192] = np.broadcast_to(b64[None, :], (8, 256)).reshape(-1)
    cconst = (Wfc2.astype(np.float64) @ fb + bfc.astype(np.float64)).astype(f32)

    return {
        "w8j": w8j.astype(NPF16), "w16j": w16j.astype(NPF16),
        "w32j": w32j.astype(NPF16), "w64w": np.ascontiguousarray(w64w.astype(NPF16)),
        "wfc2t": wfc2t.astype(NPF16),
        "wfc1t4": wfc1t4,
        "cconst": np.ascontiguousarray(cconst.reshape(1, OUT) if os.environ.get("K_CC32")
                                       else cconst.reshape(1, OUT).astype(NPF16)),
    }


# ---------------------------------------------------------------------------
# device program
# ---------------------------------------------------------------------------

def build_program(repeat=1, trace_sim=False):
    nc = bacc.Bacc("TRN2", target_bir_lowering=False, debug=False)

    dram = dict(
        x4=nc.dram_tensor("x4", [F, W], FP32, kind="ExternalInput"),
        w8j=nc.dram_tensor("w8j", [64, 256], FP16, kind="ExternalInput"),
        w16j=nc.dram_tensor("w16j", [64, 1024], FP16, kind="ExternalInput"),
        w32j=nc.dram_tensor("w32j", [64, 4096], FP16, kind="ExternalInput"),
        w64w=nc.dram_tensor("w64w", [64, 16384], FP16, kind="ExternalInput"),
        wfc2t=nc.dram_tensor("wfc2t", [64, 128, OUT], FP16, kind="ExternalInput"),
        wfc1t4=nc.dram_tensor("wfc1t4", [64, NUTT * OUT], FP32, kind="ExternalInput"),
        cconst=nc.dram_tensor("cconst", [1, OUT], FP32 if os.environ.get("K_CC32") else FP16, kind="ExternalInput"),
        out=nc.dram_tensor("out", [W, OUT], FP32, kind="ExternalOutput"),
        featsflat=nc.dram_tensor("featsflat", [64, 128, NUTT], FP16),
    )

    with tile.TileContext(nc, trace_sim=trace_sim) as tc:
        for rep in range(repeat):
            with ExitStack() as ctx:
                _emit(nc, tc, ctx, dram, rep)

    nc.compile()
    return nc


def _emit(nc, tc, ctx, dram, rep):
    if os.environ.get("K_ALL_SYNC"):
        class _S:
            dma_start = staticmethod(nc.sync.dma_start)
        scalar_dma = sync_dma = gpsimd_dma = nc.sync.dma_start
    else:
        scalar_dma = nc.scalar.dma_start
        gpsimd_dma = nc.gpsimd.dma_start
        sync_dma = nc.sync.dma_start
    const = ctx.enter_context(tc.tile_pool(name=f"const{rep}", bufs=1))
    stg = ctx.enter_context(tc.tile_pool(name=f"stg{rep}", bufs=2))
    wfc2p = ctx.enter_context(tc.tile_pool(name=f"wfc2p{rep}", bufs=2))
    outp = ctx.enter_context(tc.tile_pool(name=f"outp{rep}", bufs=2))
    ps = ctx.enter_context(tc.tile_pool(name=f"ps{rep}", bufs=2, space="PSUM"))
    psc = ctx.enter_context(tc.tile_pool(name=f"psc{rep}", bufs=1, space="PSUM"))
    psf = ctx.enter_context(tc.tile_pool(name=f"psf{rep}", bufs=2, space="PSUM"))

    CH = 8  # wfc2 k-tiles per streamed chunk

    # ---- input loads. Rings: sync = wfc2 stream; scalar = x4/w64w/out;
    # gpsimd (SWDGE) = small weights, scatters/gathers.
    x4 = const.tile([65, W], FP32, tag="x4")
    scalar_dma(x4[0:64, :], dram["x4"].ap())
    nc.vector.memset(x4[64:65, :], 1.0)
    # fp16 copy of x, duplicated into both 64-partition halves (so operands
    # can sit at base partition 0 or 64 to match w64w's j-parity halves)
    x4h = const.tile([64, W], FP16, tag="x4h")
    nc.vector.tensor_copy(x4h[0:64, :], x4[0:64, :])

    w64w = const.tile([64, 16384], FP16, tag="w64w")
    scalar_dma(w64w[:], dram["w64w"].ap())
    w8j = const.tile([64, 256], FP16, tag="w8j")
    gpsimd_dma(w8j[:], dram["w8j"].ap())
    w16j = const.tile([64, 1024], FP16, tag="w16j")
    gpsimd_dma(w16j[:], dram["w16j"].ap())
    w32j = const.tile([64, 4096], FP16, tag="w32j")
    gpsimd_dma(w32j[:], dram["w32j"].ap())
    cconst = const.tile([1, OUT], FP16 if not os.environ.get("K_CC32") else FP32, tag="cconst")
    gpsimd_dma(cconst[:], dram["cconst"].ap())
    ones1 = const.tile([1, NUTT], FP16 if not os.environ.get("K_CC32") else FP32, tag="ones1")
    nc.vector.memset(ones1[:], 1.0)

    rhs65 = const.tile([65, NUTT * OUT], FP32, tag="rhs65")
    scalar_dma(rhs65[0:64, :], dram["wfc1t4"].ap())

    feats = const.tile([128, 64 * NUTT], FP16, tag="feats")
    cps = psc.tile([NUTT, OUT], FP32, tag="cps")
    featsflat = dram["featsflat"]

    def cmms(b):
        """C matmuls for k-tile block b (16 kts = 2 chunks of CH)."""
        for ch in (2 * b, 2 * b + 1):
            chunk = wfc2p.tile([128, CH * OUT], FP16, tag="wfc2chunk")
            sync_dma(
                chunk[:],
                bass.AP(tensor=dram["wfc2t"], offset=ch * CH * 128 * OUT,
                        ap=[[OUT, 128], [128 * OUT, CH], [1, OUT]]))
            for i in range(CH):
                kt = ch * CH + i
                nc.tensor.matmul(cps[:], feats[:, kt * NUTT:(kt + 1) * NUTT],
                                 chunk[:, i * OUT:(i + 1) * OUT],
                                 start=(kt == 0), stop=False)

    def gather(b):
        gpsimd_dma(
            feats[:, b * 16 * NUTT:(b + 1) * 16 * NUTT],
            bass.AP(tensor=featsflat, offset=b * 16 * 128 * NUTT,
                    ap=[[NUTT, 128], [128 * NUTT, 16], [1, NUTT]]))

    # ---- scale h=8: 8 MMs K=64 M=32 N=256 -> PSUM (k*4+o, u*64+p)
    x8 = x4h[0:64, :].rearrange("i (u p j) -> i u p j", u=NUTT, j=8)
    acc = ps.tile([32, NUTT * 64], FP32, tag="featps")
    for j in range(8):
        nc.tensor.matmul(acc[:], w8j[:, j * 32:(j + 1) * 32], x8[:, :, :, j],
                         start=(j == 0), stop=(j == 7))
    st = stg.tile([32, NUTT * 64], FP16, tag="f8st")
    nc.vector.tensor_copy(st[:], acc[:])
    # scatter (q, u*64+p) -> featsflat[p//4, (p%4)*32+q, u]
    gpsimd_dma(
        bass.AP(tensor=featsflat, offset=0,
                ap=[[NUTT, 32], [1, NUTT], [128 * NUTT, 16], [32 * NUTT, 4]]),
        st[:].rearrange("q (u ph pl) -> q u ph pl", u=NUTT, ph=16))
    gather(0)
    cmms(0)

    # ---- scale h=16: 16 MMs K=64 M=64 N=128 -> PSUM (k*16+o, u*32+p)
    x16 = x4h[0:64, :].rearrange("i (u p j) -> i u p j", u=NUTT, j=16)
    acc = ps.tile([64, NUTT * 32], FP32, tag="featps")
    for j in range(16):
        nc.tensor.matmul(acc[:], w16j[:, j * 64:(j + 1) * 64], x16[:, :, :, j],
                         start=(j == 0), stop=(j == 15))
    st = stg.tile([64, NUTT * 32], FP16, tag="f16st")
    nc.vector.tensor_copy(st[:], acc[:])
    # scatter (q, u*32+p) -> featsflat[16+p//2, (p%2)*64+q, u]
    gpsimd_dma(
        bass.AP(tensor=featsflat, offset=16 * 128 * NUTT,
                ap=[[NUTT, 64], [1, NUTT], [128 * NUTT, 16], [64 * NUTT, 2]]),
        st[:].rearrange("q (u ph pl) -> q u ph pl", u=NUTT, ph=16))
    gather(1)
    cmms(1)

    # ---- scale h=32: 32 MMs K=64 M=128 N=64 -> PSUM (k*64+o, u*16+p)
    x32 = x4h[0:64, :].rearrange("i (u p j) -> i u p j", u=NUTT, j=32)
    acc = ps.tile([128, NUTT * 16], FP32, tag="featps")
    for j in range(32):
        nc.tensor.matmul(acc[:], w32j[:, j * 128:(j + 1) * 128], x32[:, :, :, j],
                         start=(j == 0), stop=(j == 31))
    st = stg.tile([128, NUTT * 16], FP16, tag="f32st")
    nc.vector.tensor_copy(st[:], acc[:])
    # scatter (q, u*16+p) -> featsflat[32+p, q, u]
    gpsimd_dma(
        bass.AP(tensor=featsflat, offset=32 * 128 * NUTT,
                ap=[[NUTT, 128], [1, NUTT], [128 * NUTT, 16]]),
        st[:].rearrange("q (u p) -> q u p", u=NUTT))
    gather(2)
    cmms(2)

    # ---- scale h=64: 64 MMs K=64 M=32 N=256 (x stationary, w64 streamed)
    acc = ps.tile([NUTT * 8, 256], FP32, tag="featps")
    x64 = x4h[0:64, :].rearrange("i (u p j) -> i u p j", u=NUTT, j=64)
    for j in range(64):
        nc.tensor.matmul(acc[:], x64[:, :, :, j],
                         w64w[:, j * 256:(j + 1) * 256],
                         start=(j == 0), stop=(j == 63))
    st = stg.tile([NUTT * 8, 256], FP16, tag="f64st")
    nc.vector.tensor_copy(st[:], acc[:])
    # scatter (u*8+p, o) -> featsflat[48+p*2+o//128, o%128, u]
    for u in range(NUTT):
        gpsimd_dma(
            bass.AP(tensor=featsflat, offset=48 * 128 * NUTT + u,
                    ap=[[2 * 128 * NUTT, 8], [128 * NUTT, 2], [NUTT, 128]]),
            st[u * 8:(u + 1) * 8, :].rearrange("p (g o) -> p g o", g=2))
    gather(3)
    cmms(3)

    # ---- finish C: + cconst, stage, write into rhs65 row 64
    nc.tensor.matmul(cps[:], ones1[:], cconst[:], start=False, stop=True)
    csb = stg.tile([NUTT, OUT], FP32, tag="csb")
    nc.vector.tensor_copy(csb[:], cps[:])
    for u in range(NUTT):
        gpsimd_dma(rhs65[64:65, u * OUT:(u + 1) * OUT], csb[u:u + 1, :])

    # ---- frames matmul: out rows = x^T @ Wfc1^T + 1*(C[u]+cconst)
    for u in range(NUTT):
        fsb = outp.tile([128, 4 * OUT], FP32, tag="framesout")
        for tc_i in range(4):
            fps = psf.tile([128, OUT], FP32, tag="framesps")
            nc.tensor.matmul(
                fps[:],
                x4[:, u * T + tc_i * 128: u * T + (tc_i + 1) * 128],
                rhs65[:, u * OUT:(u + 1) * OUT], start=True, stop=True)
            nc.vector.tensor_copy(fsb[:, tc_i * OUT:(tc_i + 1) * OUT], fps[:])
        scalar_dma(
            bass.AP(tensor=dram["out"], offset=u * T * OUT,
                    ap=[[OUT, 128], [128 * OUT, 4], [1, OUT]]),
            fsb[:])


_NC_CACHE = None


def _get_nc():
    global _NC_CACHE
    if _NC_CACHE is None:
        _NC_CACHE = build_program()
    return _NC_CACHE


# ---------------------------------------------------------------------------
# entry point
# ---------------------------------------------------------------------------

def run(inputs, trace=False, **kw):
    nc = _get_nc()
    prep = host_prep(inputs["W8"], inputs["b8"], inputs["W16"], inputs["b16"],
                     inputs["W32"], inputs["b32"], inputs["W64"], inputs["b64"],
                     inputs["Wfc"], inputs["bfc"])
    batch = np.asarray(inputs["batch"], np.float32)
    in_maps = []
    for c in range(NCORES):
        x4 = np.ascontiguousarray(
            batch[NUTT * c:NUTT * (c + 1)].transpose(1, 0, 2).reshape(F, W))
        m = dict(prep)
        m["x4"] = x4
        in_maps.append(m)
    res = run_bass_kernel_spmd(nc, in_maps, core_ids=list(range(NCORES)),
                               trace=trace, **kw)
    out = np.concatenate([r["out"] for r in res.results], axis=0)
    return out, res


def kernel(**inputs):
    out, _ = run(inputs)
    return out



# revision 11
# speedup vs baseline: 1.7476x; 1.7476x over previous
"""Trainium2 Bass kernel for nn_CNNModel_82222853915196.

Model (per utterance x: (64, 512)):
  multiscale patch features (h in {8,16,32,64}) -> feats (8192,)
  out[t, :] = Wfc @ concat([x[:, t], feats]) + bfc

Factorization: feats is broadcast over t, so
  out = x.T @ Wfc1.T  +  1 * (Wfc2 @ feats + cconst).T
with Wfc1 = Wfc[:, :64], Wfc2 = Wfc[:, 64:], all feature-bias terms folded
into cconst on the host.

Feature contraction: masked stationary weights over the full 64-row
contraction (rows outside [k, k+h) zeroed host-side) fuse all row offsets k
into the matmul M dim; two within-row offsets j are fused into K=128 via a
second, one-column-shifted copy of x in SBUF partitions 64..127.

All PSUM feature tiles use an [(u, p) | q] orientation so the scatter to the
DRAM feats buffer is one contiguous-run DMA per scale; the gather back to
the [feature | utt] layout needed by the C matmul is one hardware (xbar)
transposing DMA per scale.

The big fc weight (Wfc2, 6.5MB) is sharded across the 8 cores by OUTPUT
column (50 of 400 per core): per-scale feats regions are AllGather'd
(16KB -> 128KB each), every core computes C[all 32 utts, its 50 outputs]
with full-width matmuls, and one small AllToAll (3.2KB blocks) hands each
core C[its 4 utts, all 400 outputs]. cconst rides in a second ones-row of
the frames matmul (K=66).

Everything runs in fp16 except PSUM accumulation (fp32); output is written
fp16 and cast to fp32 on the host. Overall rel err ~1e-3 vs tolerance 2e-2.

Sharding: batch data parallel (32 utts -> 8 cores x 4) for everything
except the fc-weight stream, which is output-sharded as above.
"""

import os
import sys
from contextlib import ExitStack

import numpy as np

for _p in ("/opt/trn_rl_repo", "/root/.axon_site/_ro/trn_rl_repo"):
    if os.path.isdir(_p) and _p not in sys.path:
        sys.path.insert(0, _p)

import concourse.bass as bass
import concourse.tile as tile
from concourse import bacc, mybir
from concourse.bass_utils import run_bass_kernel_spmd

NCORES = 8
NUTT = 4                 # utterances per core
T = 512
F = 64
OUT = 400
OSH = OUT // NCORES      # 50, per-core output shard of the C matmul
W = NUTT * T             # 2048, free width of the x tile
FP32 = mybir.dt.float32
FP16 = mybir.dt.float16
NPF16 = np.float16


# ---------------------------------------------------------------------------
# host-side weight preparation
# ---------------------------------------------------------------------------

def _build_devindex():
    """dev[kt, fp] = reference flat feature index m in [0, 8192).

    Device feats layout (scale regions of 16 kt each):
      h=8 : kt = ph,        fp = pl*32 + k*4 + o    (p = ph*4 + pl)
      h=16: kt = 16 + ph,   fp = pl*64 + k*16 + o   (p = ph*2 + pl)
      h=32: kt = 32 + p,    fp = k*64 + o
      h=64: kt = 48+p*2+oh, fp = ol                 (o = oh*128 + ol)
    """
    dev = np.full((64, 128), -1, dtype=np.int64)
    for ph in range(16):
        for pl in range(4):
            for k in range(8):
                for o in range(4):
                    dev[ph, pl * 32 + k * 4 + o] = (k * 64 + ph * 4 + pl) * 4 + o
    for ph in range(16):
        for pl in range(2):
            for k in range(4):
                for o in range(16):
                    dev[16 + ph, pl * 64 + k * 16 + o] = \
                        2048 + (k * 32 + ph * 2 + pl) * 16 + o
    for p in range(16):
        for k in range(2):
            for o in range(64):
                dev[32 + p, k * 64 + o] = 4096 + (k * 16 + p) * 64 + o
    for p in range(8):
        for o in range(256):
            dev[48 + p * 2 + o // 128, o % 128] = 6144 + p * 256 + o
    assert dev.min() >= 0
    return dev


def _masked2(Wh, nk, h, no):
    """w2[(jo, r), j0*(nk*no) + k*no + o] = Wh[k, o, (r-k)*h + 2*j0 + jo]
    for 0 <= r-k < h, else 0."""
    f32 = np.float32
    w = np.zeros((2, 64, h // 2, nk * no), dtype=f32)
    for k in range(nk):
        Wk = np.asarray(Wh[k], f32).reshape(no, h, h)      # [o, i, j]
        for jo in range(2):
            # [i, j0, o]
            w[jo, k:k + h, :, k * no:(k + 1) * no] = \
                Wk[:, :, jo::2].transpose(1, 2, 0)
    return w.reshape(128, (h // 2) * nk * no)


def host_prep(W8, b8, W16, b16, W32, b32, W64, b64, Wfc, bfc):
    f32 = np.float32
    W64 = np.asarray(W64, f32)
    Wfc = np.asarray(Wfc, f32)
    b8 = np.asarray(b8, f32); b16 = np.asarray(b16, f32)
    b32 = np.asarray(b32, f32); b64 = np.asarray(b64, f32)
    bfc = np.asarray(bfc, f32)

    w8j2 = _masked2(W8, 8, 8, 4)        # [128, 128]
    w16j2 = _masked2(W16, 4, 16, 16)    # [128, 512]
    w32j2 = _masked2(W32, 2, 32, 64)    # [128, 2048]
    # w64w2[(jo,i), j0*256+o] = W64[o, i*64 + 2*j0 + jo]
    w64w2 = np.ascontiguousarray(
        W64.reshape(256, 64, 32, 2).transpose(3, 1, 2, 0).reshape(128, 8192))

    dev = _build_devindex()
    Wfc2 = Wfc[:, 64:]
    # wfc2kt[kt, fp, o] = Wfc2[o, dev[kt, fp]]
    wfc2kt = Wfc2[:, dev.reshape(-1)].T.reshape(64, 128, OUT)

    wfc1t4 = np.ascontiguousarray(np.tile(Wfc[:, :64].T, (1, NUTT)))

    fb = np.zeros(8192, dtype=np.float64)
    fb[0:2048] = np.broadcast_to(b8[:, None, :], (8, 64, 4)).reshape(-1)
    fb[2048:4096] = np.broadcast_to(b16[:, None, :], (4, 32, 16)).reshape(-1)
    fb[4096:6144] = np.broadcast_to(b32[:, None, :], (2, 16, 64)).reshape(-1)
    fb[6144:8192] = np.broadcast_to(b64[None, :], (8, 256)).reshape(-1)
    cconst = (Wfc2.astype(np.float64) @ fb + bfc.astype(np.float64)).astype(f32)

    return {
        "w8j2": w8j2.astype(NPF16), "w16j2": w16j2.astype(NPF16),
        "w32j2": w32j2.astype(NPF16), "w64w2": w64w2.astype(NPF16),
        "wfc2kt": wfc2kt,            # fp32, per-core shard cut in run()
        "wfc1t4": np.ascontiguousarray(wfc1t4.astype(NPF16)),
        "cconst4": np.ascontiguousarray(
            np.tile(cconst.reshape(1, OUT), (1, NUTT)).astype(NPF16)),
    }


# ---------------------------------------------------------------------------
# device program
# ---------------------------------------------------------------------------

def build_program(trace_sim=False):
    nc = bacc.Bacc("TRN2", target_bir_lowering=False, debug=False)

    dram = dict(
        xdup=nc.dram_tensor("xdup", [128, W], FP16, kind="ExternalInput"),
        w8j2=nc.dram_tensor("w8j2", [128, 128], FP16, kind="ExternalInput"),
        w16j2=nc.dram_tensor("w16j2", [128, 512], FP16, kind="ExternalInput"),
        w32j2=nc.dram_tensor("w32j2", [128, 2048], FP16, kind="ExternalInput"),
        w64w2=nc.dram_tensor("w64w2", [128, 8192], FP16, kind="ExternalInput"),
        wfc2os=nc.dram_tensor("wfc2os", [128, 64 * OSH], FP16, kind="ExternalInput"),
        wfc1t4=nc.dram_tensor("wfc1t4", [64, NUTT * OUT], FP16, kind="ExternalInput"),
        cconst4=nc.dram_tensor("cconst4", [1, NUTT * OUT], FP16, kind="ExternalInput"),
        out=nc.dram_tensor("out", [NUTT, 128, 4 * OUT], FP16, kind="ExternalOutput"),
        featsflat=nc.dram_tensor("featsflat", [256, 128], FP16),
        cpart=nc.dram_tensor("cpart", [32, OSH], FP16),
        crecv=nc.dram_tensor("crecv", [32, OSH], FP16),
    )

    with tile.TileContext(nc, trace_sim=trace_sim) as tc:
        with ExitStack() as ctx:
            _emit(nc, tc, ctx, dram)

    nc.compile()
    return nc


def _emit(nc, tc, ctx, dram):
    scalar_dma = nc.scalar.dma_start
    gpsimd_dma = nc.gpsimd.dma_start
    sync_dma = nc.sync.dma_start
    GROUPS = [list(range(NCORES))]

    const = ctx.enter_context(tc.tile_pool(name="const", bufs=1))
    stg = ctx.enter_context(tc.tile_pool(name="stg", bufs=2))
    outp = ctx.enter_context(tc.tile_pool(name="outp", bufs=2))
    dpool = ctx.enter_context(tc.tile_pool(name="dpool", bufs=1, space="DRAM"))
    ps = ctx.enter_context(tc.tile_pool(name="ps", bufs=2, space="PSUM"))
    psc = ctx.enter_context(tc.tile_pool(name="psc", bufs=1, space="PSUM"))
    psf = ctx.enter_context(tc.tile_pool(name="psf", bufs=2, space="PSUM"))

    featsall = [dpool.tile([512, 128], FP16, tag=f"featsall{b}",
                           addr_space="Shared", name=f"featsall{b}")
                for b in range(4)]
    featsflat = dram["featsflat"]

    # ---- queue layout:
    #  sync   : xdup, w64 halves, xbar gathers, C-row load
    #  scalar : wfc2os, wfc1t4, cconst4, scatters, out writes
    #  gpsimd : small weights, collectives (4x AllGather, AllToAll), cpart
    #  vector : memsets, x66, stage casts, frames casts (even)
    xdup = const.tile([128, W], FP16, tag="xdup")
    sync_dma(xdup[:], dram["xdup"].ap())

    w64w2 = const.tile([128, 8192], FP16, tag="w64w2")
    sync_dma(w64w2[:, 0:4096],
             bass.AP(tensor=dram["w64w2"], offset=0, ap=[[8192, 128], [1, 4096]]))

    wfc2os = const.tile([128, 64 * OSH], FP16, tag="wfc2os")
    scalar_dma(wfc2os[:], dram["wfc2os"].ap())

    w8j2 = const.tile([128, 128], FP16, tag="w8j2")
    gpsimd_dma(w8j2[:], dram["w8j2"].ap())
    w16j2 = const.tile([128, 512], FP16, tag="w16j2")
    gpsimd_dma(w16j2[:], dram["w16j2"].ap())
    w32j2 = const.tile([128, 2048], FP16, tag="w32j2")
    gpsimd_dma(w32j2[:], dram["w32j2"].ap())

    # frames stationary: rows 0..63 = x, rows 64,65 = ones
    x66 = const.tile([66, W], FP16, tag="x66")
    nc.vector.tensor_copy(x66[0:64, :], xdup[0:64, :])
    nc.vector.memset(x66[64:66, :], 1.0)

    # frames moving: rows 0..63 = wfc1, row 64 = C_u, row 65 = cconst
    rhs66 = const.tile([66, NUTT * OUT], FP16, tag="rhs66")
    scalar_dma(rhs66[0:64, :], dram["wfc1t4"].ap())
    scalar_dma(rhs66[65:66, :], dram["cconst4"].ap())

    feats = const.tile([128, 2048], FP16, tag="feats")
    cps = psc.tile([32, OSH], FP32, tag="cps")

    def gather(b):
        """xbar-transpose gathered region b -> feats[:, b*512:(b+1)*512]."""
        nc.sync.dma_start_transpose(
            feats[:, b * 512:(b + 1) * 512], featsall[b][:, :])

    def allgather(b):
        nc.gpsimd.collective_compute(
            "AllGather", mybir.AluOpType.bypass, replica_groups=GROUPS,
            ins=[bass.AP(tensor=featsflat, offset=b * 64 * 128,
                         ap=[[128, 64], [1, 128]]).opt()],
            outs=[featsall[b].opt()])

    def cmms(b):
        """C matmuls for scale region b: 16 kts, M=32 utts, N=50."""
        fv = feats.rearrange("f (s c u k) -> f s c u k", s=4, c=8, u=4)
        for k in range(16):
            kt = b * 16 + k
            nc.tensor.matmul(cps[:], fv[:, b, :, :, k],
                             wfc2os[:, kt * OSH:(kt + 1) * OSH],
                             start=(kt == 0), stop=(kt == 63))

    xv = xdup[:].rearrange("i (u t) -> i u t", u=NUTT)

    # ---- scale h=8: 8 MMs K=128 M=128(u2,ph,pl) N=32; two u-halves
    for half in range(2):
        acc = ps.tile([128, 32], FP32, tag="featps")
        x8 = xv[:, 2 * half:2 * half + 2, :].rearrange(
            "i u (p j) -> i u p j", j=8)
        for j0 in range(4):
            nc.tensor.matmul(acc[:], x8[:, :, :, 2 * j0],
                             w8j2[:, j0 * 32:(j0 + 1) * 32],
                             start=(j0 == 0), stop=(j0 == 3))
        st = stg.tile([128, 32], FP16, tag="featst")
        nc.vector.tensor_copy(st[:], acc[:])
        scalar_dma(
            bass.AP(tensor=featsflat, offset=half * 2 * 2048,
                    ap=[[32, 128], [1, 32]]),
            st[:])
    allgather(0)
    gather(0)
    cmms(0)

    # ---- scale h=16: 8 MMs K=128 M=128(u,p32) N=64
    acc = ps.tile([128, 64], FP32, tag="featps")
    x16 = xv.rearrange("i u (p j) -> i u p j", j=16)
    for j0 in range(8):
        nc.tensor.matmul(acc[:], x16[:, :, :, 2 * j0],
                         w16j2[:, j0 * 64:(j0 + 1) * 64],
                         start=(j0 == 0), stop=(j0 == 7))
    st = stg.tile([128, 64], FP16, tag="featst")
    nc.vector.tensor_copy(st[:], acc[:])
    scalar_dma(
        bass.AP(tensor=featsflat, offset=64 * 128, ap=[[64, 128], [1, 64]]),
        st[:])
    allgather(1)
    gather(1)
    cmms(1)

    # second half of w64 behind gather(0)/gather(1) on the sync queue
    sync_dma(w64w2[:, 4096:8192],
             bass.AP(tensor=dram["w64w2"], offset=4096,
                     ap=[[8192, 128], [1, 4096]]))

    # ---- scale h=32: 16 MMs K=128 M=64(u,p16) N=128
    acc = ps.tile([64, 128], FP32, tag="featps")
    x32 = xv.rearrange("i u (p j) -> i u p j", j=32)
    for j0 in range(16):
        nc.tensor.matmul(acc[:], x32[:, :, :, 2 * j0],
                         w32j2[:, j0 * 128:(j0 + 1) * 128],
                         start=(j0 == 0), stop=(j0 == 15))
    st = stg.tile([64, 128], FP16, tag="featst")
    nc.vector.tensor_copy(st[:], acc[:])
    scalar_dma(
        bass.AP(tensor=featsflat, offset=128 * 128, ap=[[128, 64], [1, 128]]),
        st[:])
    allgather(2)
    gather(2)
    cmms(2)

    # ---- scale h=64: 32 MMs K=128 M=32(u,p8) N=256
    acc = ps.tile([32, 256], FP32, tag="featps")
    x64 = xv.rearrange("i u (p j) -> i u p j", j=64)
    for j0 in range(32):
        nc.tensor.matmul(acc[:], x64[:, :, :, 2 * j0],
                         w64w2[:, j0 * 256:(j0 + 1) * 256],
                         start=(j0 == 0), stop=(j0 == 31))
    st = stg.tile([32, 256], FP16, tag="featst")
    nc.vector.tensor_copy(st[:], acc[:])
    scalar_dma(
        bass.AP(tensor=featsflat, offset=192 * 128, ap=[[256, 32], [1, 256]]),
        st[:])
    allgather(3)
    gather(3)
    cmms(3)

    # ---- redistribute C: [32 utts, my 50 outs] -> [my 4 utts, 400 outs]
    csb = stg.tile([32, OSH], FP16, tag="csb")
    nc.vector.tensor_copy(csb[:], cps[:])
    gpsimd_dma(dram["cpart"].ap(), csb[:])
    nc.gpsimd.collective_compute(
        "AllToAll", mybir.AluOpType.bypass, replica_groups=GROUPS,
        ins=[dram["cpart"].ap().opt()], outs=[dram["crecv"].ap().opt()])
    # crecv[(p, u), ol] -> rhs66 row 64 cols (u, p*50 + ol)
    sync_dma(
        rhs66[64:65, :].rearrange("z (u p ol) -> z u p ol", u=NUTT, p=NCORES),
        bass.AP(tensor=dram["crecv"], offset=0,
                ap=[[32 * OSH, 1], [OSH, NUTT], [NUTT * OSH, NCORES], [1, OSH]]))

    # ---- frames matmul: out rows = x^T @ Wfc1^T + 1*C[u] + 1*cconst
    for u in range(NUTT):
        fsb = outp.tile([128, 4 * OUT], FP16, tag="framesout")
        for tc_i in range(4):
            fps = psf.tile([128, OUT], FP32, tag="framesps")
            nc.tensor.matmul(
                fps[:],
                x66[:, u * T + tc_i * 128: u * T + (tc_i + 1) * 128],
                rhs66[:, u * OUT:(u + 1) * OUT], start=True, stop=True)
            if tc_i % 2 == 0:
                nc.vector.tensor_copy(fsb[:, tc_i * OUT:(tc_i + 1) * OUT], fps[:])
            else:
                nc.scalar.activation(fsb[:, tc_i * OUT:(tc_i + 1) * OUT], fps[:],
                                     mybir.ActivationFunctionType.Copy)
            scalar_dma(
                bass.AP(tensor=dram["out"],
                        offset=u * 128 * 4 * OUT + tc_i * OUT,
                        ap=[[4 * OUT, 128], [1, OUT]]),
                fsb[:, tc_i * OUT:(tc_i + 1) * OUT])


_NC_CACHE = None


def _get_nc():
    global _NC_CACHE
    if _NC_CACHE is None:
        _NC_CACHE = build_program()
    return _NC_CACHE


# ---------------------------------------------------------------------------
# entry point
# ---------------------------------------------------------------------------

def run(inputs, trace=False, **kw):
    nc = _get_nc()
    prep = host_prep(inputs["W8"], inputs["b8"], inputs["W16"], inputs["b16"],
                     inputs["W32"], inputs["b32"], inputs["W64"], inputs["b64"],
                     inputs["Wfc"], inputs["bfc"])
    wfc2kt = prep.pop("wfc2kt")          # [64, 128, OUT] fp32
    batch = np.asarray(inputs["batch"], np.float32)
    in_maps = []
    for c in range(NCORES):
        x4 = batch[NUTT * c:NUTT * (c + 1)].transpose(1, 0, 2).reshape(F, W)
        xdup = np.zeros((128, W), dtype=NPF16)
        xdup[0:64] = x4.astype(NPF16)
        xdup[64:128, :W - 1] = x4[:, 1:].astype(NPF16)
        m = dict(prep)
        m["xdup"] = xdup
        # per-core output shard of wfc2: [128 fp, 64 kt * 50]
        m["wfc2os"] = np.ascontiguousarray(
            wfc2kt[:, :, OSH * c:OSH * (c + 1)].transpose(1, 0, 2)
            .reshape(128, 64 * OSH).astype(NPF16))
        in_maps.append(m)
    res = run_bass_kernel_spmd(nc, in_maps, core_ids=list(range(NCORES)),
                               trace=trace, **kw)
    # out[u, t, tc, o] -> rows u*512 + tc*128 + t
    out = np.concatenate(
        [r["out"].reshape(NUTT, 128, 4, OUT).transpose(0, 2, 1, 3)
         .reshape(NUTT * T, OUT) for r in res.results], axis=0)
    return out.astype(np.float32), res


def kernel(**inputs):
    out, _ = run(inputs)
    return out


# revision 12
# speedup vs baseline: 3.1791x; 1.8191x over previous
"""Trainium2 Bass kernel for nn_CNNModel_82222853915196.

Model (per utterance x: (64, 512)):
  multiscale patch features (h in {8,16,32,64}) -> feats (8192,)
  out[t, :] = Wfc @ concat([x[:, t], feats]) + bfc

Factorization: feats is broadcast over t, so
  out = x.T @ Wfc1.T  +  1 * (Wfc2 @ feats + cconst).T
with Wfc1 = Wfc[:, :64], Wfc2 = Wfc[:, 64:], all feature-bias terms folded
into cconst on the host.

Feature contraction: masked stationary weights over the full 64-row
contraction (rows outside [k, k+h) zeroed host-side) fuse all row offsets k
into the matmul M dim; additionally two within-row offsets j are fused into
K=128 by keeping a second copy of x (shifted left by one column) in SBUF
partitions 64..127, halving the number of feature matmuls.

All PSUM feature tiles use an [(u, p) | q] orientation so the scatter to the
DRAM feats buffer is one contiguous-run DMA per scale; the gather back to
the [feature | (u, kt)] layout needed by the C matmul is a single hardware
(xbar) transposing DMA per scale. wfc2 is pre-permuted host-side to match,
so its 6.5MB stream is fully contiguous per partition row.

Everything runs in fp16 except PSUM accumulation (fp32); output is written
fp16 and cast to fp32 on the host. Overall rel err ~1e-3 vs tolerance 2e-2.

Sharding: pure data parallel - 32 utterances -> 8 cores x 4. Weights
replicated; no cross-core communication.
"""

import os
import sys
from contextlib import ExitStack

import numpy as np

for _p in ("/opt/trn_rl_repo", "/root/.axon_site/_ro/trn_rl_repo"):
    if os.path.isdir(_p) and _p not in sys.path:
        sys.path.insert(0, _p)

import concourse.bass as bass
import concourse.tile as tile
from concourse import bacc, mybir
from concourse.bass_utils import run_bass_kernel_spmd

NCORES = 8
NUTT = 4                 # utterances per core
T = 512
F = 64
OUT = 400
W = NUTT * T             # 2048, free width of the x tile
FP32 = mybir.dt.float32
FP16 = mybir.dt.float16
NPF16 = np.float16


# ---------------------------------------------------------------------------
# host-side weight preparation
# ---------------------------------------------------------------------------

def _build_devindex():
    """dev[kt, fp] = reference flat feature index m in [0, 8192).

    Device feats layout (scale regions of 16 kt each, 4 u per kt):
      h=8 : kt = ph,        fp = pl*32 + k*4 + o    (p = ph*4 + pl)
      h=16: kt = 16 + ph,   fp = pl*64 + k*16 + o   (p = ph*2 + pl)
      h=32: kt = 32 + p,    fp = k*64 + o
      h=64: kt = 48+p*2+oh, fp = ol                 (o = oh*128 + ol)
    """
    dev = np.full((64, 128), -1, dtype=np.int64)
    for ph in range(16):
        for pl in range(4):
            for k in range(8):
                for o in range(4):
                    dev[ph, pl * 32 + k * 4 + o] = (k * 64 + ph * 4 + pl) * 4 + o
    for ph in range(16):
        for pl in range(2):
            for k in range(4):
                for o in range(16):
                    dev[16 + ph, pl * 64 + k * 16 + o] = \
                        2048 + (k * 32 + ph * 2 + pl) * 16 + o
    for p in range(16):
        for k in range(2):
            for o in range(64):
                dev[32 + p, k * 64 + o] = 4096 + (k * 16 + p) * 64 + o
    for p in range(8):
        for o in range(256):
            dev[48 + p * 2 + o // 128, o % 128] = 6144 + p * 256 + o
    assert dev.min() >= 0
    return dev


def _masked2(Wh, nk, h, no):
    """w2[(jo, r), j0*(nk*no) + k*no + o] = Wh[k, o, (r-k)*h + 2*j0 + jo]
    for 0 <= r-k < h, else 0."""
    f32 = np.float32
    w = np.zeros((2, 64, h // 2, nk * no), dtype=f32)
    for k in range(nk):
        Wk = np.asarray(Wh[k], f32).reshape(no, h, h)      # [o, i, j]
        for jo in range(2):
            # [i, j0, o]
            w[jo, k:k + h, :, k * no:(k + 1) * no] = \
                Wk[:, :, jo::2].transpose(1, 2, 0)
    return w.reshape(128, (h // 2) * nk * no)


def host_prep(W8, b8, W16, b16, W32, b32, W64, b64, Wfc, bfc):
    f32 = np.float32
    W64 = np.asarray(W64, f32)
    Wfc = np.asarray(Wfc, f32)
    b8 = np.asarray(b8, f32); b16 = np.asarray(b16, f32)
    b32 = np.asarray(b32, f32); b64 = np.asarray(b64, f32)
    bfc = np.asarray(bfc, f32)

    w8j2 = _masked2(W8, 8, 8, 4)        # [128, 128]
    w16j2 = _masked2(W16, 4, 16, 16)    # [128, 512]
    w32j2 = _masked2(W32, 2, 32, 64)    # [128, 2048]
    # w64w2[(jo,i), j0*256+o] = W64[o, i*64 + 2*j0 + jo]
    w64w2 = np.ascontiguousarray(
        W64.reshape(256, 64, 32, 2).transpose(3, 1, 2, 0).reshape(128, 8192))

    dev = _build_devindex()
    Wfc2 = Wfc[:, 64:]
    # wfc2c[fp, kt*OUT + o] = Wfc2[o, dev[kt, fp]]
    wfc2c = np.ascontiguousarray(
        Wfc2[:, dev.reshape(-1)].T.reshape(64, 128, OUT)
        .transpose(1, 0, 2).reshape(128, 64 * OUT))

    wfc1t4 = np.ascontiguousarray(np.tile(Wfc[:, :64].T, (1, NUTT)))

    fb = np.zeros(8192, dtype=np.float64)
    fb[0:2048] = np.broadcast_to(b8[:, None, :], (8, 64, 4)).reshape(-1)
    fb[2048:4096] = np.broadcast_to(b16[:, None, :], (4, 32, 16)).reshape(-1)
    fb[4096:6144] = np.broadcast_to(b32[:, None, :], (2, 16, 64)).reshape(-1)
    fb[6144:8192] = np.broadcast_to(b64[None, :], (8, 256)).reshape(-1)
    cconst = (Wfc2.astype(np.float64) @ fb + bfc.astype(np.float64)).astype(f32)

    return {
        "w8j2": w8j2.astype(NPF16), "w16j2": w16j2.astype(NPF16),
        "w32j2": w32j2.astype(NPF16), "w64w2": w64w2.astype(NPF16),
        "wfc2c": wfc2c.astype(NPF16),
        "wfc1t4": np.ascontiguousarray(wfc1t4.astype(NPF16)),
        "cconst": np.ascontiguousarray(cconst.reshape(1, OUT).astype(NPF16)),
    }


# ---------------------------------------------------------------------------
# device program
# ---------------------------------------------------------------------------

def build_program(trace_sim=False):
    nc = bacc.Bacc("TRN2", target_bir_lowering=False, debug=False)

    dram = dict(
        xdup=nc.dram_tensor("xdup", [128, W], FP16, kind="ExternalInput"),
        w8j2=nc.dram_tensor("w8j2", [128, 128], FP16, kind="ExternalInput"),
        w16j2=nc.dram_tensor("w16j2", [128, 512], FP16, kind="ExternalInput"),
        w32j2=nc.dram_tensor("w32j2", [128, 2048], FP16, kind="ExternalInput"),
        w64w2=nc.dram_tensor("w64w2", [128, 8192], FP16, kind="ExternalInput"),
        wfc2c=nc.dram_tensor("wfc2c", [128, 64 * OUT], FP16, kind="ExternalInput"),
        wfc1t4=nc.dram_tensor("wfc1t4", [64, NUTT * OUT], FP16, kind="ExternalInput"),
        cconst=nc.dram_tensor("cconst", [1, OUT], FP16, kind="ExternalInput"),
        out=nc.dram_tensor("out", [NUTT, 128, 4 * OUT], FP16, kind="ExternalOutput"),
        featsflat=nc.dram_tensor("featsflat", [256, 128], FP16),
    )

    with tile.TileContext(nc, trace_sim=trace_sim) as tc:
        with ExitStack() as ctx:
            _emit(nc, tc, ctx, dram)

    nc.compile()
    return nc


def _emit(nc, tc, ctx, dram):
    scalar_dma = nc.scalar.dma_start
    gpsimd_dma = nc.gpsimd.dma_start
    sync_dma = nc.sync.dma_start

    const = ctx.enter_context(tc.tile_pool(name="const", bufs=1))
    stg = ctx.enter_context(tc.tile_pool(name="stg", bufs=2))
    wfc2p = ctx.enter_context(tc.tile_pool(name="wfc2p", bufs=2))
    outp = ctx.enter_context(tc.tile_pool(name="outp", bufs=2))
    ps = ctx.enter_context(tc.tile_pool(name="ps", bufs=2, space="PSUM"))
    psc = ctx.enter_context(tc.tile_pool(name="psc", bufs=1, space="PSUM"))
    psf = ctx.enter_context(tc.tile_pool(name="psf", bufs=3, space="PSUM"))

    CH = 16  # wfc2 kt per streamed chunk (one chunk per scale region)

    # ---- input loads. sync = wfc2c stream (8 chunks, issued upfront);
    # scalar = xdup / gathers / w64w2 quarters / out; gpsimd = small
    # weights + scatters.
    xdup = const.tile([128, W], FP16, tag="xdup")
    scalar_dma(xdup[:], dram["xdup"].ap())

    w64w2 = const.tile([128, 8192], FP16, tag="w64w2")

    def load_wfc2_chunk(ch, dma):
        chunk = wfc2p.tile([128, CH * OUT], FP16, tag="wfc2chunk", bufs=4)
        dma(chunk[:],
            bass.AP(tensor=dram["wfc2c"], offset=ch * CH * OUT,
                    ap=[[64 * OUT, 128], [1, CH * OUT]]))
        return chunk

    # sync: xdup, w64 halves (feature path, needed first), chunks 0, 2
    # scalar: chunks 1, 3, then gathers/csb/outs
    for hi in range(2):
        sync_dma(w64w2[:, hi * 4096:(hi + 1) * 4096],
                 bass.AP(tensor=dram["w64w2"], offset=hi * 4096,
                         ap=[[8192, 128], [1, 4096]]))
    chunks = [None] * 4
    chunks[0] = load_wfc2_chunk(0, sync_dma)
    chunks[1] = load_wfc2_chunk(1, scalar_dma)
    chunks[2] = load_wfc2_chunk(2, sync_dma)
    chunks[3] = load_wfc2_chunk(3, scalar_dma)

    w8j2 = const.tile([128, 128], FP16, tag="w8j2")
    gpsimd_dma(w8j2[:], dram["w8j2"].ap())
    w16j2 = const.tile([128, 512], FP16, tag="w16j2")
    gpsimd_dma(w16j2[:], dram["w16j2"].ap())
    w32j2 = const.tile([128, 2048], FP16, tag="w32j2")
    gpsimd_dma(w32j2[:], dram["w32j2"].ap())
    cconst = const.tile([1, OUT], FP16, tag="cconst")
    gpsimd_dma(cconst[:], dram["cconst"].ap())
    ones1 = const.tile([1, NUTT], FP16, tag="ones1")
    nc.vector.memset(ones1[:], 1.0)

    # frames stationary: rows 0..63 = x, row 64 = ones
    x65 = const.tile([65, W], FP16, tag="x65")
    nc.vector.tensor_copy(x65[0:64, :], xdup[0:64, :])
    nc.vector.memset(x65[64:65, :], 1.0)

    # frames moving: rows 0..63 = wfc1 (per-utt tiled), row 64 = C_u + cconst
    rhs65 = const.tile([65, NUTT * OUT], FP16, tag="rhs65")
    scalar_dma(rhs65[0:64, :], dram["wfc1t4"].ap())

    feats = const.tile([128, 256], FP16, tag="feats")
    cps = psc.tile([NUTT, OUT], FP32, tag="cps")
    featsflat = dram["featsflat"]

    def cmms(b):
        """C matmuls for scale region b (16 kts = chunk b)."""
        fv = feats.rearrange("f (s u k) -> f s u k", s=4, u=4)
        chunk = chunks[b]
        for i in range(CH):
            kt = b * CH + i              # global kt in [0, 64)
            nc.tensor.matmul(cps[:], fv[:, b, :, i],
                             chunk[:, i * OUT:(i + 1) * OUT],
                             start=(kt == 0), stop=False)

    def gather(b):
        """xbar-transpose DRAM region b -> feats[:, b*64:(b+1)*64]."""
        nc.scalar.dma_start_transpose(
            feats[:, b * 64:(b + 1) * 64],
            bass.AP(tensor=featsflat, offset=b * 64 * 128,
                    ap=[[128, 64], [1, 128]]))

    xv = xdup[:].rearrange("i (u t) -> i u t", u=NUTT)

    # ---- scale h=8: 8 MMs K=128 M=128(u2,ph,pl) N=32; two u-halves
    # PSUM [(u2, ph, pl), k*4+o]
    for half in range(2):
        acc = ps.tile([128, 32], FP32, tag=f"ps8{half}")
        x8 = xv[:, 2 * half:2 * half + 2, :].rearrange(
            "i u (p j) -> i u p j", j=8)
        for j0 in range(4):
            nc.tensor.matmul(acc[:], x8[:, :, :, 2 * j0],
                             w8j2[:, j0 * 32:(j0 + 1) * 32],
                             start=(j0 == 0), stop=(j0 == 3))
        st = stg.tile([128, 32], FP16, tag=f"st8{half}")
        nc.vector.tensor_copy(st[:], acc[:])
        gpsimd_dma(
            bass.AP(tensor=featsflat, offset=half * 2 * 2048,
                    ap=[[32, 128], [1, 32]]),
            st[:])
    gather(0)
    cmms(0)

    # ---- scale h=16: 8 MMs K=128 M=128(u,p32) N=64
    acc = ps.tile([128, 64], FP32, tag="ps16")
    x16 = xv.rearrange("i u (p j) -> i u p j", j=16)
    for j0 in range(8):
        nc.tensor.matmul(acc[:], x16[:, :, :, 2 * j0],
                         w16j2[:, j0 * 64:(j0 + 1) * 64],
                         start=(j0 == 0), stop=(j0 == 7))
    st = stg.tile([128, 64], FP16, tag="st16")
    nc.vector.tensor_copy(st[:], acc[:])
    gpsimd_dma(
        bass.AP(tensor=featsflat, offset=64 * 128,
                ap=[[64, 128], [1, 64]]),
        st[:])
    gather(1)
    cmms(1)

    # ---- scale h=32: 16 MMs K=128 M=64(u,p16) N=128
    acc = ps.tile([64, 128], FP32, tag="ps32")
    x32 = xv.rearrange("i u (p j) -> i u p j", j=32)
    for j0 in range(16):
        nc.tensor.matmul(acc[:], x32[:, :, :, 2 * j0],
                         w32j2[:, j0 * 128:(j0 + 1) * 128],
                         start=(j0 == 0), stop=(j0 == 15))
    st = stg.tile([64, 128], FP16, tag="st32")
    nc.vector.tensor_copy(st[:], acc[:])
    gpsimd_dma(
        bass.AP(tensor=featsflat, offset=128 * 128,
                ap=[[128, 64], [1, 128]]),
        st[:])
    gather(2)
    cmms(2)

    # ---- scale h=64: 32 MMs K=128 M=32(u,p8) N=256
    acc = ps.tile([32, 256], FP32, tag="ps64")
    x64 = xv.rearrange("i u (p j) -> i u p j", j=64)
    for j0 in range(32):
        nc.tensor.matmul(acc[:], x64[:, :, :, 2 * j0],
                         w64w2[:, j0 * 256:(j0 + 1) * 256],
                         start=(j0 == 0), stop=(j0 == 31))
    st = stg.tile([32, 256], FP16, tag="st64")
    nc.vector.tensor_copy(st[:], acc[:])
    gpsimd_dma(
        bass.AP(tensor=featsflat, offset=192 * 128,
                ap=[[256, 32], [1, 256]]),
        st[:])
    gather(3)
    cmms(3)

    # ---- finish C: + cconst, stage fp16, write into rhs65 row 64
    nc.tensor.matmul(cps[:], ones1[:], cconst[:], start=False, stop=True)
    csb = stg.tile([NUTT, OUT], FP16, tag="csb")
    nc.vector.tensor_copy(csb[:], cps[:])
    for u in range(NUTT):
        sync_dma(rhs65[64:65, u * OUT:(u + 1) * OUT], csb[u:u + 1, :])

    # ---- frames matmul: out rows = x^T @ Wfc1^T + 1*(C[u]+cconst)
    for u in range(NUTT):
        fsb = outp.tile([128, 4 * OUT], FP16, tag="framesout")
        for tc_i in range(4):
            fps = psf.tile([128, OUT], FP32, tag="framesps")
            nc.tensor.matmul(
                fps[:],
                x65[:, u * T + tc_i * 128: u * T + (tc_i + 1) * 128],
                rhs65[:, u * OUT:(u + 1) * OUT], start=True, stop=True)
            if tc_i % 2 == 0:
                nc.vector.tensor_copy(fsb[:, tc_i * OUT:(tc_i + 1) * OUT], fps[:])
            else:
                nc.scalar.activation(fsb[:, tc_i * OUT:(tc_i + 1) * OUT], fps[:],
                                     mybir.ActivationFunctionType.Copy)
            scalar_dma(
                bass.AP(tensor=dram["out"],
                        offset=u * 128 * 4 * OUT + tc_i * OUT,
                        ap=[[4 * OUT, 128], [1, OUT]]),
                fsb[:, tc_i * OUT:(tc_i + 1) * OUT])


_NC_CACHE = None


def _get_nc():
    global _NC_CACHE
    if _NC_CACHE is None:
        _NC_CACHE = build_program()
    return _NC_CACHE


# ---------------------------------------------------------------------------
# entry point
# ---------------------------------------------------------------------------

def run(inputs, trace=False, **kw):
    nc = _get_nc()
    prep = host_prep(inputs["W8"], inputs["b8"], inputs["W16"], inputs["b16"],
                     inputs["W32"], inputs["b32"], inputs["W64"], inputs["b64"],
                     inputs["Wfc"], inputs["bfc"])
    batch = np.asarray(inputs["batch"], np.float32)
    in_maps = []
    for c in range(NCORES):
        x4 = batch[NUTT * c:NUTT * (c + 1)].transpose(1, 0, 2).reshape(F, W)
        xdup = np.zeros((128, W), dtype=NPF16)
        xdup[0:64] = x4.astype(NPF16)
        xdup[64:128, :W - 1] = x4[:, 1:].astype(NPF16)
        m = dict(prep)
        m["xdup"] = xdup
        in_maps.append(m)
    res = run_bass_kernel_spmd(nc, in_maps, core_ids=list(range(NCORES)),
                               trace=trace, **kw)
    # out[u, t, tc, o] -> rows u*512 + tc*128 + t
    out = np.concatenate(
        [r["out"].reshape(NUTT, 128, 4, OUT).transpose(0, 2, 1, 3)
         .reshape(NUTT * T, OUT) for r in res.results], axis=0)
    return out.astype(np.float32), res


def kernel(**inputs):
    out, _ = run(inputs)
    return out
